# revision 1
# baseline (speedup 1.0000x reference)
"""Trainium2 Bass kernel for per-sample-routed ConvTranspose1d (Dereverb T60
decoder).

Math: for each sample b with routed weight W (Cin=512, K=16), stride 8, pad 8:
    y[t] = A[p, m+1] + A[p+8, m]   where t = 8m + p (p in [0,8), m in [0,3999)),
    A[k, q] = sum_ci W[ci, k] * x[ci, q]        (a 16x512 @ 512x4000 matmul)

Sharding: pure data parallel, B=16 -> 2 samples on each of 8 NeuronCores.
Routing (t60 -> 1 of 41 kernels) is a host-side gather of 32KB per sample.

Design (bf16, DMA-bound):
  - host converts x and the routed W to bf16; W is pre-packed into the exact
    SBUF layout w40[p, c, col] (taps 0..7 at cols 0..7, taps 8..15 at cols
    32..39, zeros elsewhere) so the device load is one clean DMA.
  - x loads as ONE DMA per sample ([128, 4, 4000] bf16, 8KB runs per
    (partition, chunk) row) on the sync ring, which x owns exclusively --
    measured: sharing that queue with w, splitting across rings, or
    host-sequential HBM layout are all slower.
  - 8 j-tiles of 500 output columns; each tile's matmul computes A for 501
    q-columns (500*j .. 500*j+500 inclusive), so the shifted pair-add
    z[p, m] = ps[p, m+1-j0] + ps[32+p, m-j0] is self-contained per tile
    (both operands from the same PSUM tile, no cross-tile boundary work).
  - bf16 matmul runs at 1 cycle/row (fp32 is 4); PSUM accumulation is fp32.
  - engine ops can read only ONE PSUM operand, so the pair-add is staged:
    ACT (scalar) copies the hi taps psum->z, DVE adds the shifted lo taps.
  - no PE transposes / staging copies: z[8, 3999] bf16 is DMA'd out
    contiguously and the final p-interleave y[8m+p] = z[p, m] is a cheap
    host-side reshape. This removes ~4000 32-byte DMA packets per sample
    that made the baseline DMA-bound at 275 GB/s.
Steady state: DMA ~97% busy (~8.4 MB/rep at ~400 GB/s), Tensor ~85% duty
(17.9 us/rep = bf16 floor), slope ~21.1 us/rep vs 20.2 us measured pure-DMA
floor. The residual gap is power throttling: with compute running, the core
spends ~30% of time at a 0.5 utilization cap (dma-only runs show zero
throttle), so less engine work per byte is the only lever left.
fp8 x fails the 2e-2 gate (measured 2.7e-2 even with bf16 W); shipping raw
A taps to the host loses (keeps tensor power, adds ~0.9 us of DMA).
"""
import numpy as np
import ml_dtypes

import concourse.bass as bass
import concourse.tile as tile
from concourse import bacc, mybir
from concourse.bass_utils import run_bass_kernel_spmd

B, CIN, L, KSZ = 16, 512, 4000, 16
LOUT = (L - 1) * 8 - 2 * 8 + KSZ  # 31992
NCORES = 8
PER = B // NCORES                 # 2 samples per core
NCHUNK = CIN // 128               # 4
JW = 500                          # j-tile output width
NJ = 8
MV = L - 1                        # 3999 valid output m positions
F32 = mybir.dt.float32
BF16 = mybir.dt.bfloat16

_CACHE = {}


def _build(reps=1, mode="full", xbufs=4, pabufs=8, zbufs=3, wbufs=3):
    # x loads as ONE DMA per sample: [128, 4, 4000] bf16, 8KB contiguous
    # runs per (partition, chunk) row. Bigger packets -> ~420 GB/s vs
    # ~326 GB/s with 2KB runs, and 1 issue instead of 4 on the Sync ring
    # (each issue measured ~2us).
    nc = bacc.Bacc("TRN2", target_bir_lowering=False, debug=False,
                   num_devices=NCORES)
    x = nc.dram_tensor("x", [PER, CIN, L], BF16, kind="ExternalInput").ap()
    w = nc.dram_tensor("w", [PER, 128, NCHUNK * 40], BF16,
                       kind="ExternalInput").ap()
    y = nc.dram_tensor("y", [PER, 8, MV], BF16, kind="ExternalOutput").ap()

    with tile.TileContext(nc) as tc:
        with tc.tile_pool(name="xp", bufs=xbufs) as xp, \
             tc.tile_pool(name="wp", bufs=wbufs) as wp, \
             tc.tile_pool(name="zp", bufs=zbufs) as zp, \
             tc.tile_pool(name="pa", bufs=pabufs, space="PSUM") as pa:

            for rep in range(reps):
                for s in range(PER):
                    w40 = wp.tile([128, NCHUNK, 40], BF16, tag="w40")
                    nc.scalar.dma_start(
                        w40[:], w[s].rearrange("p (c k) -> p c k", c=NCHUNK))

                    z = zp.tile([8, L], BF16, tag="z")
                    xt = xp.tile([128, NCHUNK, L], BF16, tag="xt")
                    nc.sync.dma_start(
                        xt[:], x[s].rearrange("(c p) l -> p c l", p=128))
                    for j in range(NJ):
                        j0 = JW * j
                        if mode == "dmaonly":
                            continue
                        nq = min(JW + 1, L - j0)   # 501, last tile 500
                        ps = pa.tile([40, JW + 1], F32, tag="pa")
                        for c in range(NCHUNK):
                            nc.tensor.matmul(
                                ps[:, 0:nq], w40[:, c, :],
                                xt[:, c, j0: j0 + nq],
                                start=(c == 0), stop=(c == NCHUNK - 1))
                        nm = min(JW, MV - j0)      # 500, last tile 499
                        # TT can read only one PSUM operand: stage the hi
                        # taps into z on ACT, then add the shifted lo
                        # taps on Vector (one PSUM input each).
                        nc.scalar.copy(
                            z[0:8, j0: j0 + nm], ps[32:40, 0: nm])
                        nc.vector.tensor_tensor(
                            z[0:8, j0: j0 + nm],
                            z[0:8, j0: j0 + nm],
                            ps[0:8, 1: 1 + nm],
                            mybir.AluOpType.add)

                    if mode == "dmaonly":
                        zd = zp.tile([8, L], BF16, tag="zd")
                        nc.vector.memset(zd[:], 0.0)
                        nc.scalar.dma_start(y[s], zd[0:8, 0:MV])
                    else:
                        nc.scalar.dma_start(y[s], z[0:8, 0:MV])

    nc.compile()
    return nc


def _route(t60s):
    idx = np.round(t60s.astype(np.float32) * np.float32(100.0))
    return np.tile(idx.astype(np.int32), 2) - 10  # (B,)


def get_nc(reps=1, f32r=False, mode="full"):
    key = (reps, mode)
    if key not in _CACHE:
        _CACHE[key] = _build(reps=reps, mode=mode)
    return _CACHE[key]


def make_in_maps(input, t60s, kernel_weight):
    idx = _route(np.asarray(t60s))
    wg = np.asarray(kernel_weight)[idx, :, 0, :]      # (B, Cin, K) fp32
    # pack into the SBUF w40 layout: [p, c, col] with taps 0..7 at cols 0..7
    # and taps 8..15 at cols 32..39 (base partitions {0,32} for engine ops).
    w40 = np.zeros((B, 128, NCHUNK, 40), dtype=ml_dtypes.bfloat16)
    wr = wg.reshape(B, NCHUNK, 128, KSZ)              # ci = c*128 + p
    w40[:, :, :, 0:8] = wr.transpose(0, 2, 1, 3)[:, :, :, 0:8]
    w40[:, :, :, 32:40] = wr.transpose(0, 2, 1, 3)[:, :, :, 8:16]
    xin = np.asarray(input, dtype=np.float32).astype(ml_dtypes.bfloat16)
    in_maps = []
    for c in range(NCORES):
        sl = slice(PER * c, PER * (c + 1))
        in_maps.append({
            "x": np.ascontiguousarray(xin[sl]),
            "w": np.ascontiguousarray(
                w40[sl].reshape(PER, 128, NCHUNK * 40)),
        })
    return in_maps


def _run(input, t60s, kernel_weight, trace=False):
    nc = get_nc()
    in_maps = make_in_maps(input, t60s, kernel_weight)
    res = run_bass_kernel_spmd(nc, in_maps, core_ids=list(range(NCORES)),
                               trace=trace)
    out = np.empty((B, 1, LOUT), dtype=np.float32)
    for c in range(NCORES):
        yr = res.results[c]["y"]                      # (PER, 8, MV) bf16
        for s in range(PER):
            # y[8m+p] = yr[s, p, m]
            out[PER * c + s, 0, :] = np.ascontiguousarray(
                yr[s].T).reshape(-1)[:LOUT]
    return out, res


def kernel(input, t60s, kernel_weight):
    out, _ = _run(input, t60s, kernel_weight, trace=False)
    return out



# revision 5
# speedup vs baseline: 1.1766x; 1.1766x over previous
"""Trainium2 Bass kernel for per-sample-routed ConvTranspose1d (Dereverb T60
decoder).

Math: for each sample b with routed weight W (Cin=512, K=16), stride 8, pad 8:
    y[t] = A[p, m+1] + A[p+8, m]   where t = 8m + p (p in [0,8), m in [0,3999)),
    A[k, q] = sum_ci W[ci, k] * x[ci, q]        (a 16x512 @ 512x4000 matmul)

Sharding: pure data parallel, B=16 -> 2 samples on each of 8 NeuronCores.
Routing (t60 -> 1 of 41 kernels) is a host-side gather of 32KB per sample.

Design (fp8 DoubleRow, single-shot-latency focused):
  - x is sent as e4m3 (halves the DMA bytes vs bf16: 2.05MB/sample).  The
    host computes S[m] = sum_c (x - fp8(x))[c, m] and adds the rank-1
    quantization-error compensation  y += wbar_lo[q]*S[m+1] + wbar_hi[q]*S[m]
    (wbar = channel-mean of W) to the returned output in fp32.  Measured
    end-to-end rel err 1.31e-2 (vs 2.71e-2 uncorrected, gate 2e-2).
  - W is sent as fp8(W) plus the fp8 residual fp8(W - fp8(W)); the device
    accumulates both parts in PSUM, recovering ~bf16 weight precision.
  - matmuls run in DoubleRow perf mode (both operands e4m3, 0.5 cycles per
    moving column, 256-channel contraction per instruction): 4 passes
    (2 k-pair tiles x 2 W parts) x 8 j-tiles per sample.  Pass-outer order
    reuses the loaded stationary across all 8 j-tiles.
  - x DMA is split per k-pair half ([128, 8000] e4m3, 8KB contiguous runs)
    so passes for k-pair 0 start after only 1.02MB has landed -- compute
    overlaps the x DMA instead of waiting for the full sample.
  - PSUM: 8 tiles [40, 501] fp32 (one bank each) accumulate across the 4
    passes; taps 0..7 at psum partitions 0..7, taps 8..15 at 32..39 (engine
    ops reading PSUM need base partition 0/32).
  - pair-add stays as before: ACT copies the hi taps psum->z, DVE adds the
    shifted lo taps (each engine op reads only one PSUM operand); z[8, 3999]
    bf16 goes out contiguously and the host does the p-interleave reshape.
"""
import numpy as np
import ml_dtypes

import concourse.bass as bass
import concourse.tile as tile
from concourse import bacc, mybir
from concourse.bass_utils import run_bass_kernel_spmd

B, CIN, L, KSZ = 16, 512, 4000, 16
LOUT = (L - 1) * 8 - 2 * 8 + KSZ  # 31992
NCORES = 8
PER = B // NCORES                 # 2 samples per core
JW = 500                          # j-tile output width
NJ = 8
MV = L - 1                        # 3999 valid output m positions
F32 = mybir.dt.float32
BF16 = mybir.dt.bfloat16
F8 = mybir.dt.float8e4
NPF8 = ml_dtypes.float8_e4m3     # matches mybir.dt.float8e4 on device

_CACHE = {}


def _build(reps=1, mode="full", xbufs=3, pabufs=1, zbufs=3, wbufs=3):
    nc = bacc.Bacc("TRN2", target_bir_lowering=False, debug=False,
                   num_devices=NCORES)
    # x[s, p, t, i*L + l] = fp8(x)[s, c, l], c = t*256 + i*128 + p
    x = nc.dram_tensor("x", [PER, 128, 2, 2 * L], F8,
                       kind="ExternalInput").ap()
    # w[s, p, ((t*2+P)*2+i)*48 + col]; cols 0..7 = taps 0..7, 32..39 = taps
    # 8..15, zeros elsewhere; P=0 -> fp8(W), P=1 -> fp8(W - fp8(W))
    w = nc.dram_tensor("w", [PER, 128, 2 * 2 * 2 * 48], F8,
                       kind="ExternalInput").ap()
    y = nc.dram_tensor("y", [PER, 8, MV], BF16, kind="ExternalOutput").ap()

    DR = mybir.MatmulPerfMode.DoubleRow

    with tile.TileContext(nc) as tc:
        with tc.tile_pool(name="xp", bufs=xbufs) as xp, \
             tc.tile_pool(name="wp", bufs=wbufs) as wp, \
             tc.tile_pool(name="zp", bufs=zbufs) as zp, \
             tc.tile_pool(name="pa", bufs=pabufs, space="PSUM") as pa:

            for rep in range(reps):
                for s in range(PER):
                    wt = wp.tile([128, 2, 2, 2, 48], F8, tag="wt")
                    nc.scalar.dma_start(
                        wt[:],
                        w[s].rearrange("p (t P i k) -> p t P i k",
                                       t=2, P=2, i=2))

                    z = zp.tile([8, L], BF16, tag="z")
                    xt = xp.tile([128, 2, 2, L], F8, tag="xt")
                    # split by k-pair so pass t=0 can start after half the
                    # sample's x has landed
                    for t in range(2):
                        nc.sync.dma_start(
                            xt[:, t],
                            x[s, :, t].rearrange("p (i l) -> p i l", i=2))

                    if mode == "dmaonly":
                        zd = zp.tile([8, L], BF16, tag="zd")
                        nc.vector.memset(zd[:], 0.0)
                        nc.scalar.dma_start(y[s], zd[0:8, 0:MV])
                        continue

                    ps = [pa.tile([48, JW + 1], F32, tag=f"pa{j}",
                                  name=f"ps{j}")
                          for j in range(NJ)]
                    nqs = [min(JW + 1, L - JW * j) for j in range(NJ)]
                    for pi, (t, P) in enumerate(
                            [(0, 0), (0, 1), (1, 0), (1, 1)]):
                        for j in range(NJ):
                            j0 = JW * j
                            nq = nqs[j]
                            nc.tensor.matmul(
                                ps[j][:, 0:nq],
                                wt[:, t, P],           # [128, 2(i), 48]
                                xt[:, t, :, j0: j0 + nq],  # [128, 2(i), nq]
                                start=(pi == 0), stop=(pi == 3),
                                perf_mode=DR)
                    for j in range(NJ):
                        j0 = JW * j
                        nm = min(JW, MV - j0)
                        nc.scalar.copy(
                            z[0:8, j0: j0 + nm], ps[j][32:40, 0: nm])
                        nc.vector.tensor_tensor(
                            z[0:8, j0: j0 + nm],
                            z[0:8, j0: j0 + nm],
                            ps[j][0:8, 1: 1 + nm],
                            mybir.AluOpType.add)

                    nc.scalar.dma_start(y[s], z[0:8, 0:MV])

    nc.compile()
    return nc


def _route(t60s):
    idx = np.round(t60s.astype(np.float32) * np.float32(100.0))
    return np.tile(idx.astype(np.int32), 2) - 10  # (B,)


def get_nc(reps=1, f32r=False, mode="full"):
    key = (reps, mode)
    if key not in _CACHE:
        _CACHE[key] = _build(reps=reps, mode=mode)
    return _CACHE[key]


def _pack(input, t60s, kernel_weight):
    idx = _route(np.asarray(t60s))
    wg = np.asarray(kernel_weight, dtype=np.float32)[idx, :, 0, :]  # (B,512,16)
    w8 = wg.astype(NPF8)
    wr8 = (wg - w8.astype(np.float32)).astype(NPF8)
    # w_pack[b, p, t, P, i, col]: c = t*256 + i*128 + p
    w_pack = np.zeros((B, 128, 2, 2, 2, 48), dtype=NPF8)
    for part, wq in enumerate((w8, wr8)):
        wv = wq.reshape(B, 2, 2, 128, KSZ)        # (b, t, i, p, k)
        wv = wv.transpose(0, 3, 1, 2, 4)          # (b, p, t, i, k)
        w_pack[:, :, :, part, :, 0:8] = wv[..., 0:8]
        w_pack[:, :, :, part, :, 32:40] = wv[..., 8:16]

    xin = np.asarray(input, dtype=np.float32)
    x8 = xin.astype(NPF8)
    # rank-1 quantization-error compensation, applied host-side after gather
    S = (xin - x8.astype(np.float32)).sum(axis=1)          # (B, L) fp32
    wbar = wg.mean(axis=1)                                  # (B, 16) fp32
    # x_pack[b, p, t, i*L + l] = x8[b, t*256 + i*128 + p, l]
    x_pack = np.ascontiguousarray(
        x8.reshape(B, 2, 2, 128, L).transpose(0, 3, 1, 2, 4)
        .reshape(B, 128, 2, 2 * L))
    return x_pack, w_pack, S, wbar


def make_in_maps(input, t60s, kernel_weight):
    x_pack, w_pack, _, _ = _pack(input, t60s, kernel_weight)
    in_maps = []
    for c in range(NCORES):
        sl = slice(PER * c, PER * (c + 1))
        in_maps.append({
            "x": np.ascontiguousarray(x_pack[sl]),
            "w": np.ascontiguousarray(
                w_pack[sl].reshape(PER, 128, 2 * 2 * 2 * 48)),
        })
    return in_maps


def _run(input, t60s, kernel_weight, trace=False):
    nc = get_nc()
    x_pack, w_pack, S, wbar = _pack(input, t60s, kernel_weight)
    in_maps = []
    for c in range(NCORES):
        sl = slice(PER * c, PER * (c + 1))
        in_maps.append({
            "x": np.ascontiguousarray(x_pack[sl]),
            "w": np.ascontiguousarray(
                w_pack[sl].reshape(PER, 128, 2 * 2 * 2 * 48)),
        })
    res = run_bass_kernel_spmd(nc, in_maps, core_ids=list(range(NCORES)),
                               trace=trace)
    out = np.empty((B, 1, LOUT), dtype=np.float32)
    for c in range(NCORES):
        yr = res.results[c]["y"]                      # (PER, 8, MV) bf16
        for s in range(PER):
            b = PER * c + s
            zm = np.asarray(yr[s], dtype=np.float32)  # (8, MV) p-major
            # y[8m+p] = z[p, m] + wbar_lo[p]*S[m+1] + wbar_hi[p]*S[m]
            zm = zm + (np.outer(wbar[b, 0:8], S[b, 1:]) +
                       np.outer(wbar[b, 8:16], S[b, :-1]))
            out[b, 0, :] = np.ascontiguousarray(zm.T).reshape(-1)[:LOUT]
    return out, res


def kernel(input, t60s, kernel_weight):
    out, _ = _run(input, t60s, kernel_weight, trace=False)
    return out


# revision 6
# speedup vs baseline: 1.3921x; 1.1832x over previous
"""Trainium2 Bass kernel for per-sample-routed ConvTranspose1d (Dereverb T60
decoder).

Math: for each sample b with routed weight W (Cin=512, K=16), stride 8, pad 8:
    y[t] = A[p, m+1] + A[p+8, m]   where t = 8m + p (p in [0,8), m in [0,3999)),
    A[k, q] = sum_ci W[ci, k] * x[ci, q]        (a 16x512 @ 512x4000 matmul)

Sharding: pure data parallel, B=16 -> 2 samples on each of 8 NeuronCores.
Routing (t60 -> 1 of 41 kernels) is a host-side gather of 32KB per sample.

Design (fp8 DoubleRow, single-shot-latency focused):
  - x is sent as e4m3 (halves DMA bytes vs bf16).  W is centered (V = W - mu,
    mu = per-tap channel mean) and sent as fp8(V) in stationary cols 0..15
    plus the fp8 residual fp8(V - fp8(V)) in cols 16..31: the PE computes
    both output groups from ONE moving x stream (output columns are free),
    so weight-precision recovery costs no extra passes.
  - DoubleRow perf mode contracts 256 channels/instruction at the same
    column rate as bf16 => 2 passes (k-pair t=0,1) x 8 j-tiles x 500 cols
    per sample ~ 3.8us/sample of PE time (vs 13.5us bf16).
  - The tap pair-add and interleave move to the HOST: the device only copies
    whole psum tiles [32, 500] to SBUF (one op per tile, alternating
    Scalar/Vector engines) and DMAs A out.  Host assembles in fp32:
        A = A_V + A_Vres + mu_k * colsum(x)[m]   (the mu term also absorbs
        the rank-1 fp8-quantization-error compensation exactly), then
        y[8m+p] = A[p, m+1] + A[p+8, m].
    Measured end-to-end rel err 1.31e-2 (gate 2e-2).
  - w for both samples loads first in one small DMA on the sync ring (the
    scalar ring measured ~3us of extra latency and stalled the first
    matmul); x follows split per (sample, k-pair) for DMA/compute overlap.
  - ~36 tiny warmup matmuls on the w tile run while x streams in, holding
    the PE's power ramp (p-state) up so real matmuls start at full rate
    (cold PE runs 2x slower for the first ~3us).
"""
import numpy as np
import ml_dtypes

import concourse.bass as bass
import concourse.tile as tile
from concourse import bacc, mybir
from concourse.bass_utils import run_bass_kernel_spmd

B, CIN, L, KSZ = 16, 512, 4000, 16
LOUT = (L - 1) * 8 - 2 * 8 + KSZ  # 31992
NCORES = 8
PER = B // NCORES                 # 2 samples per core
JW = 500                          # j-tile output width
NJ = 8
MV = L - 1                        # 3999 valid output m positions
NWARM = 36
F32 = mybir.dt.float32
BF16 = mybir.dt.bfloat16
F8 = mybir.dt.float8e4
NPF8 = ml_dtypes.float8_e4m3     # matches mybir.dt.float8e4 on device

_CACHE = {}


def _build(reps=1, mode="full", xbufs=2, zbufs=2, nwarm=NWARM):
    nc = bacc.Bacc("TRN2", target_bir_lowering=False, debug=False,
                   num_devices=NCORES)
    # x[s, p, t, i*L + l] = fp8(x)[s, c, l], c = t*256 + i*128 + p
    x = nc.dram_tensor("x", [PER, 128, 2, 2 * L], F8,
                       kind="ExternalInput").ap()
    # w[p, (s t i col)]: cols 0..15 = fp8(V) taps 0..15, cols 16..31 =
    # fp8(V - fp8(V)) taps 0..15; V = W - mu
    w = nc.dram_tensor("w", [128, PER * 2 * 2 * 32], F8,
                       kind="ExternalInput").ap()
    y = nc.dram_tensor("y", [PER, 32, L], BF16, kind="ExternalOutput").ap()

    DR = mybir.MatmulPerfMode.DoubleRow

    with tile.TileContext(nc) as tc:
        with tc.tile_pool(name="xp", bufs=xbufs) as xp, \
             tc.tile_pool(name="wp", bufs=1) as wp, \
             tc.tile_pool(name="zp", bufs=zbufs) as zp, \
             tc.tile_pool(name="pa", bufs=1, space="PSUM") as pa:

            for rep in range(reps):
                # both samples' weights in one small DMA, first on the ring
                wt = wp.tile([128, PER, 2, 2, 32], F8, tag="wt",
                             name=f"wt{rep}")
                nc.sync.dma_start(
                    wt[:], w.rearrange("p (s t i k) -> p s t i k",
                                       s=PER, t=2, i=2))
                xts = []
                for s in range(PER):
                    xt = xp.tile([128, 2, 2, L], F8, tag="xt",
                                 name=f"xt{s}")
                    xts.append(xt)
                    for t in range(2):
                        nc.sync.dma_start(
                            xt[:, t],
                            x[s, :, t].rearrange("p (i l) -> p i l", i=2))

                ps = [pa.tile([32, JW], F32, tag=f"pa{j}", name=f"ps{j}")
                      for j in range(NJ)]

                if mode == "dmaonly":
                    for s in range(PER):
                        zd = zp.tile([32, L], BF16, tag="z", name=f"zd{s}")
                        nc.vector.memset(zd[:], 0.0)
                        nc.scalar.dma_start(y[s], zd[:])
                    continue

                # PE p-state warmup: tiny matmuls on the w tile while x
                # streams in; ps[0] is overwritten by the first real pass
                # (start=True).
                if rep == 0:
                    for it in range(nwarm):
                        nc.tensor.matmul(
                            ps[0][:, 0:16], wt[:, 0, 0],
                            wt[:, 0, 1, :, 0:16],
                            start=True, stop=True, perf_mode=DR)

                for s in range(PER):
                    xt = xts[s]
                    z = zp.tile([32, L], BF16, tag="z", name=f"z{s}")
                    for pi in range(2):
                        for j in range(NJ):
                            j0 = JW * j
                            nc.tensor.matmul(
                                ps[j][:, 0:JW],
                                wt[:, s, pi],            # [128, 2(i), 32]
                                xt[:, pi, :, j0: j0 + JW],  # [128, 2(i), JW]
                                start=(pi == 0), stop=(pi == 1),
                                perf_mode=DR)
                            if pi == 1:
                                if j % 2 == 0:
                                    nc.scalar.copy(
                                        z[:, j0: j0 + JW], ps[j][:, 0:JW])
                                else:
                                    nc.vector.tensor_scalar_add(
                                        z[:, j0: j0 + JW], ps[j][:, 0:JW],
                                        0.0)
                    nc.scalar.dma_start(y[s], z[:])

    nc.compile()
    return nc


def _route(t60s):
    idx = np.round(t60s.astype(np.float32) * np.float32(100.0))
    return np.tile(idx.astype(np.int32), 2) - 10  # (B,)


def get_nc(reps=1, f32r=False, mode="full"):
    key = (reps, mode)
    if key not in _CACHE:
        _CACHE[key] = _build(reps=reps, mode=mode)
    return _CACHE[key]


def _pack(input, t60s, kernel_weight):
    idx = _route(np.asarray(t60s))
    wg = np.asarray(kernel_weight, dtype=np.float32)[idx, :, 0, :]  # (B,512,16)
    mu = wg.mean(axis=1)                                   # (B, 16)
    V = wg - mu[:, None, :]
    v8 = V.astype(NPF8)
    vr8 = (V - v8.astype(np.float32)).astype(NPF8)
    # w_pack[b, p, t, i, col]: c = t*256 + i*128 + p; col 0..15 -> v8 taps,
    # col 16..31 -> vr8 taps
    w_pack = np.zeros((B, 128, 2, 2, 32), dtype=NPF8)
    for part, wq in enumerate((v8, vr8)):
        wv = wq.reshape(B, 2, 2, 128, KSZ)        # (b, t, i, p, k)
        wv = wv.transpose(0, 3, 1, 2, 4)          # (b, p, t, i, k)
        w_pack[:, :, :, :, 16 * part:16 * part + 16] = wv

    xin = np.asarray(input, dtype=np.float32)
    x8 = xin.astype(NPF8)
    Tx = xin.sum(axis=1)                                   # (B, L) fp32
    # x_pack[b, p, t, i*L + l] = x8[b, t*256 + i*128 + p, l]
    x_pack = np.ascontiguousarray(
        x8.reshape(B, 2, 2, 128, L).transpose(0, 3, 1, 2, 4)
        .reshape(B, 128, 2, 2 * L))
    return x_pack, w_pack, Tx, mu


def make_in_maps(input, t60s, kernel_weight):
    x_pack, w_pack, _, _ = _pack(input, t60s, kernel_weight)
    in_maps = []
    for c in range(NCORES):
        sl = slice(PER * c, PER * (c + 1))
        # w[p, (s t i k)]
        wl = np.ascontiguousarray(
            w_pack[sl].transpose(1, 0, 2, 3, 4).reshape(128, PER * 2 * 2 * 32))
        in_maps.append({
            "x": np.ascontiguousarray(x_pack[sl]),
            "w": wl,
        })
    return in_maps


def _run(input, t60s, kernel_weight, trace=False):
    nc = get_nc()
    x_pack, w_pack, Tx, mu = _pack(input, t60s, kernel_weight)
    in_maps = make_in_maps(input, t60s, kernel_weight)
    res = run_bass_kernel_spmd(nc, in_maps, core_ids=list(range(NCORES)),
                               trace=trace)
    out = np.empty((B, 1, LOUT), dtype=np.float32)
    for c in range(NCORES):
        yr = res.results[c]["y"]                      # (PER, 32, L) bf16
        for s in range(PER):
            b = PER * c + s
            zm = np.asarray(yr[s], dtype=np.float32)  # (32, L)
            A = zm[0:16] + zm[16:32]                  # (16, L)
            A += mu[b][:, None] * Tx[b][None, :]
            # y[8m+p] = A[p, m+1] + A[p+8, m]
            ym = A[0:8, 1:] + A[8:16, :-1]            # (8, MV)
            out[b, 0, :] = np.ascontiguousarray(ym.T).reshape(-1)[:LOUT]
    return out, res


def kernel(input, t60s, kernel_weight):
    out, _ = _run(input, t60s, kernel_weight, trace=False)
    return out
